# revision 5
# baseline (speedup 1.0000x reference)
"""Deformable convolution (B=4, C=256, 64x64, COUT=256, 3x3) on 8 trn2 NeuronCores.

Sharding: data-parallel over (batch, output-row-half): core i handles batch i//2,
output rows [32*(i%2), 32*(i%2)+32). Weight replicated.

Device pipeline per core:
  1. index/fraction math from offsets (DVE, fp32, immediate-scalar ops only)
  2. one dma_gather per (tap, n-chunk) from a host-staged "quad" image Q in
     DRAM: Q[y*68+x] = [P[y,x,:], P[y+1,x,:]] (fp16, zero-padded borders), so
     each 2KB gathered element carries the full 2x2 bilinear patch
     [TL BL TR BR] for 256 channels.
  3. bilinear combine on DVE in fp16 2x mode: per-quad multiplies against
     pair-duplicated weights (innermost [1,2] AP keeps the fast mode legal)
     + 3 pairwise adds
  4. XBAR dma_start_transpose (sync engine) flips [n,c] -> [c,n] blocks;
     no PE transposes, no PSUM round trip for columns
  5. fp16 GEMM (K=2304) accumulating in PSUM, fp32 output
"""

import os
import sys

for _p in ("/root/.axon_site", "/root/.axon_site/_ro/trn_rl_repo", "/opt/trn_rl_repo"):
    if os.path.isdir(_p) and _p not in sys.path:
        sys.path.append(_p)

import numpy as np

import concourse.bass as bass
import concourse.bacc as bacc
import concourse.mybir as mybir
from concourse.tile import TileContext

# ---------------------------------------------------------------- constants
B, CIN, H, W = 4, 256, 64, 64
COUT, KH, KW = 256, 3, 3
KK = KH * KW                      # 9 taps
HO = WO = 64
HOH = 32                          # output rows per core
N = HOH * WO                      # 2048 positions per core
NJ = 16                           # 128-blocks of N
NCH = 2                           # gather chunks (h)
NJH = NJ // NCH                   # j' blocks per chunk = 8
NIDX = NJH * 128                  # 1024 indices per gather
PAD = 2                           # zero-pad border of the staged image
Hp = Wp = H + 2 * PAD             # 68
NROW = Hp * Wp                    # 4624 quad rows
QE = 4 * CIN                      # 1024 elements per gathered quad
KB = 2 * KK                       # 18 K-blocks of 128
C288 = KK * 2 * NJ                # 288
C144 = KK * NJ                    # 144
FP16 = mybir.dt.float16
FP32 = mybir.dt.float32
I16 = mybir.dt.int16
I32 = mybir.dt.int32
OP = mybir.AluOpType

_MAX_WAITS = 1


def _split_multiwait_instructions(nc):
    """This walrus build rejects >1 sync wait on one instruction ('Too many
    sync wait commands'); hoist extras onto single-wait EventSemaphore
    instructions inserted just before it."""
    fn = nc.m.functions[0]
    for bb in fn.blocks:
        new_insts = []
        for inst in bb.instructions:
            si = getattr(inst, "sync_info", None)
            if si is not None and si.on_wait and len(si.on_wait) > _MAX_WAITS:
                waits = list(si.on_wait)
                for k, w in enumerate(waits[_MAX_WAITS:]):
                    ev = mybir.InstEventSemaphore(
                        name=f"{inst.name}_wsplit{k}",
                        ins=[],
                        outs=[],
                        sync_info=mybir.SyncInfo(on_wait=[w], on_update=[]),
                    )
                    ev.engine = inst.engine
                    new_insts.append(ev)
                si.on_wait = waits[:_MAX_WAITS]
            new_insts.append(inst)
        bb.instructions[:] = new_insts


# ---------------------------------------------------------------- device kernel
def build_nc(split_waits=True):
    nc = bacc.Bacc()
    img = nc.dram_tensor("img", [NROW, QE // 2], FP16, kind="ExternalInput")
    offg = nc.dram_tensor("offg", [128, C288], FP32, kind="ExternalInput")
    grid = nc.dram_tensor("grid", [128, C288], FP32, kind="ExternalInput")
    w2 = nc.dram_tensor("w2", [128, KB * COUT], FP16, kind="ExternalInput")
    out = nc.dram_tensor("out", [COUT, N], FP32, kind="ExternalOutput")

    # gather source: rows of 2*QE fp16 with stride QE (overlapping x-pairs)
    img_src = bass.AP(img[:].tensor, 0, [[QE // 2, NROW - 1], [1, QE]])

    with TileContext(nc) as tc:
        with (
            tc.tile_pool(name="const", bufs=1) as constp,
            tc.tile_pool(name="small", bufs=1) as smallp,
            tc.tile_pool(name="gath", bufs=3) as gathp,
            tc.tile_pool(name="prod", bufs=2) as prodp,
            tc.tile_pool(name="interp", bufs=2) as vp,
            tc.tile_pool(name="cols", bufs=3) as colsp,
            tc.tile_pool(name="osb", bufs=2) as osbp,
            tc.tile_pool(name="pout", bufs=2, space="PSUM") as poutp,
        ):
            # ---- constants
            w_sb = constp.tile([128, KB, COUT], FP16)
            nc.sync.dma_start(w_sb[:], w2[:].rearrange("p (kb o) -> p kb o", o=COUT))
            offg_sb = constp.tile([128, C288], FP32)
            nc.sync.dma_start(offg_sb[:], offg[:])
            grid_sb = constp.tile([128, C288], FP32)
            nc.sync.dma_start(grid_sb[:], grid[:])

            # ---- stage A: sampling positions, fractions, weights, indices
            pp = smallp.tile([128, C288], FP32, tag="pp")
            nc.vector.tensor_tensor(out=pp[:], in0=offg_sb[:], in1=grid_sb[:],
                                    op=OP.add)
            # floor(pp): int-cast rounds-to-nearest on HW but truncates in
            # CoreSim; correct either to floor via (cast > pp) ? cast-1 : cast.
            p_i = smallp.tile([128, C288], I32, tag="pi")
            nc.vector.tensor_copy(out=p_i[:], in_=pp[:])
            p_f = smallp.tile([128, C288], FP32, tag="pf")
            nc.vector.tensor_copy(out=p_f[:], in_=p_i[:])
            gt_t = smallp.tile([128, C288], FP32, tag="gtt")
            nc.vector.tensor_tensor(out=gt_t[:], in0=p_f[:], in1=pp[:],
                                    op=OP.is_gt)
            nc.vector.tensor_tensor(out=p_f[:], in0=p_f[:], in1=gt_t[:],
                                    op=OP.subtract)

            # per-tap (k, d, j) views: y = d0, x = d1 -> [128, 9, 16]
            def yx(t):
                v4 = t[:].rearrange("p (k d j) -> p k d j", d=2, j=NJ)
                return v4[:, :, 0, :], v4[:, :, 1, :]

            pf_y, pf_x = yx(p_f)

            # indices first (the gather stream depends only on these):
            # idx = clamp(y0-14,0,67)*68 + clamp(x0-14,0,66)  (+16 host bias)
            tt_ = smallp.tile([128, C144], FP32, tag="tt")
            ss_ = smallp.tile([128, C144], FP32, tag="ss")
            t3 = tt_[:].rearrange("p (k j) -> p k j", j=NJ)
            s3 = ss_[:].rearrange("p (k j) -> p k j", j=NJ)
            nc.vector.tensor_scalar(out=t3, in0=pf_y, scalar1=-14.0,
                                    scalar2=0.0, op0=OP.add, op1=OP.max)
            nc.vector.tensor_scalar(out=tt_[:], in0=tt_[:], scalar1=67.0,
                                    scalar2=float(Wp), op0=OP.min, op1=OP.mult)
            nc.vector.tensor_scalar(out=s3, in0=pf_x, scalar1=-14.0,
                                    scalar2=0.0, op0=OP.add, op1=OP.max)
            nc.vector.tensor_scalar(out=ss_[:], in0=ss_[:], scalar1=66.0,
                                    scalar2=None, op0=OP.min)
            idxf = smallp.tile([128, C144], FP32, tag="idxf")
            nc.vector.tensor_tensor(out=idxf[:], in0=tt_[:], in1=ss_[:],
                                    op=OP.add)
            idxs = smallp.tile([128, C144], I16, tag="idxs")
            nc.vector.tensor_copy(out=idxs[:], in_=idxf[:])

            # fold [128, (k h j')] -> [16, (a k h j')]: partition group a of
            # idxs becomes a free dim (288B-run descriptors, one DMA per a)
            idxf1 = constp.tile([16, 8, C144], I16)
            for a in range(8):
                nc.sync.dma_start(out=idxf1[:, a, :],
                                  in_=idxs[a * 16:(a + 1) * 16, :])
            # DVE reorder -> [16, (k h j' a)] so each gather's 64-index slice
            # [(k h), (j' a)] is contiguous, then replicate to all 8 groups.
            idx2 = constp.tile([128, 8 * C144], I16)
            i_in = idxf1[:].rearrange("b a (kh j) -> b kh j a", kh=KK * NCH)
            i_out = idx2[0:16].rearrange("b (kh j a) -> b kh j a",
                                         kh=KK * NCH, j=NJH)
            nc.vector.tensor_copy(out=i_out, in_=i_in)
            for lo in (16, 32, 64):
                nc.sync.dma_start(out=idx2[lo:2 * lo], in_=idx2[0:lo])

            # bilinear fractions + weights (after idx: off the gather path)
            fr = smallp.tile([128, C288], FP32, tag="fr")
            nc.vector.tensor_tensor(out=fr[:], in0=pp[:], in1=p_f[:],
                                    op=OP.subtract)
            omfr = smallp.tile([128, C288], FP32, tag="omfr")
            nc.vector.tensor_scalar(out=omfr[:], in0=fr[:], scalar1=-1.0,
                                    scalar2=1.0, op0=OP.mult, op1=OP.add)
            fr_y, fr_x = yx(fr)
            om_y, om_x = yx(omfr)

            # bilinear weights -> w4 [128, (k j q)] fp16, q order (TL,BL,TR,BR)
            w4 = smallp.tile([128, C144 * 4], FP16, tag="w4")
            w4v = w4[:].rearrange("p (k j q) -> p k j q", k=KK, j=NJ)
            nc.vector.tensor_tensor(out=w4v[:, :, :, 0], in0=om_y, in1=om_x,
                                    op=OP.mult)  # TL: (1-ly)(1-lx)
            nc.vector.tensor_tensor(out=w4v[:, :, :, 1], in0=fr_y, in1=om_x,
                                    op=OP.mult)  # BL: ly(1-lx)
            nc.vector.tensor_tensor(out=w4v[:, :, :, 2], in0=om_y, in1=fr_x,
                                    op=OP.mult)  # TR: (1-ly)lx
            nc.vector.tensor_tensor(out=w4v[:, :, :, 3], in0=fr_y, in1=fr_x,
                                    op=OP.mult)  # BR: ly lx
            # pair-duplicated weights: innermost [1,2] AP keeps DVE 2x legal
            w4d = smallp.tile([128, C144 * 8], FP16, tag="w4d")
            wv = w4[:]
            w4_rep = bass.AP(wv.tensor, wv.offset,
                             [wv.ap[0], [1, C144 * 4], [0, 2]])
            nc.vector.tensor_copy(
                out=w4d[:].rearrange("p (f two) -> p f two", two=2),
                in_=w4_rep)

            # ---- stages B-E per (h, k)
            for h in range(NCH):
                pout = [poutp.tile([128, NIDX], FP32, tag=f"pout{ob}",
                                   name=f"pout{ob}_{h}")
                        for ob in range(2)]
                for k in range(KK):
                    g = gathp.tile([128, NJH, QE], FP16, tag="g")
                    nc.gpsimd.dma_gather(
                        g[:], img_src,
                        idx2[:, (k * NCH + h) * 64:(k * NCH + h + 1) * 64],
                        NIDX, NIDX, QE, elem_step=QE // 2)

                    # per-quad multiply, fp16 2x (all APs innermost stride 1)
                    prods = prodp.tile([128, NJH, 4, CIN], FP16, tag="prods")
                    g4 = g[:].rearrange("p a (q c) -> p a q c", q=4)
                    for q in range(4):
                        wd = w4d[:, k * 128 + h * 64 + q * 2:]
                        w_q = bass.AP(
                            wd.tensor, wd.offset,
                            [wd.ap[0], [8, NJH], [0, CIN // 2], [1, 2]])
                        nc.vector.tensor_tensor(out=prods[:, :, q, :],
                                                in0=g4[:, :, q, :], in1=w_q,
                                                op=OP.mult)
                    # v layout [p, cb, j', c128] so the per-cb slice is a 2D
                    # contiguous [128, 1024] block for the XBAR transpose
                    v1 = vp.tile([128, 2, NJH, 128], FP16, tag="va")
                    v2 = vp.tile([128, 2, NJH, 128], FP16, tag="vb")

                    def pq(q):
                        s = prods[:, :, q, :]
                        return s.rearrange("p a (cb c) -> p cb a c", cb=2)

                    nc.vector.tensor_tensor(out=v1[:], in0=pq(0), in1=pq(1),
                                            op=OP.add)
                    nc.vector.tensor_tensor(out=v2[:], in0=pq(2), in1=pq(3),
                                            op=OP.add)
                    nc.vector.tensor_tensor(out=v1[:], in0=v1[:], in1=v2[:],
                                            op=OP.add)

                    for cb in range(2):
                        cols = colsp.tile([128, NJH, 128], FP16, tag="cols")
                        nc.sync.dma_start_transpose(
                            out=cols[:],
                            in_=v1[:, cb].rearrange("p a c -> p (a c)"))
                        cols_f = cols[:].rearrange("p a c -> p (a c)")
                        kb = k * 2 + cb
                        for ob in range(2):
                            for ns in range(2):
                                nc.tensor.matmul(
                                    pout[ob][:, ns * 512:(ns + 1) * 512],
                                    lhsT=w_sb[:, kb, ob * 128:(ob + 1) * 128],
                                    rhs=cols_f[:, ns * 512:(ns + 1) * 512],
                                    start=(kb == 0), stop=(kb == KB - 1))

                for ob in range(2):
                    osb = osbp.tile([128, NIDX], FP32, tag="osb")
                    nc.scalar.copy(out=osb[:], in_=pout[ob][:])
                    nc.sync.dma_start(
                        out=out[ob * 128:(ob + 1) * 128,
                                h * NIDX:(h + 1) * NIDX],
                        in_=osb[:])

    nc.compile()
    if split_waits:
        _split_multiwait_instructions(nc)
    return nc


_NC_CACHE = None


def _get_nc():
    global _NC_CACHE
    if _NC_CACHE is None:
        _NC_CACHE = build_nc()
    return _NC_CACHE


# ---------------------------------------------------------------- host prep
def _prep_core_inputs(x, offset, weight):
    """Build the 8 per-core input maps (pure layout/pad/cast transforms)."""
    x = np.asarray(x, np.float32)
    offset = np.asarray(offset, np.float32)
    weight = np.asarray(weight, np.float32)

    imgs = []
    for b in range(B):
        pimg = np.zeros((Hp + 1, Wp, CIN), np.float16)
        pimg[PAD:PAD + H, PAD:PAD + W, :] = x[b].transpose(1, 2, 0)
        # quad rows: Q[y*68+x] = [P[y,x,:], P[y+1,x,:]]
        quad = np.concatenate([pimg[:Hp], pimg[1:Hp + 1]], axis=2)
        imgs.append(np.ascontiguousarray(quad.reshape(NROW, QE // 2)))

    # dense per-partition weight layout: w2[p, kb, o] = w[o, cb*128+p, kh, kw]
    wT = weight.transpose(2, 3, 1, 0).reshape(KB, 128, COUT).astype(np.float16)
    w2 = np.ascontiguousarray(wT.transpose(1, 0, 2).reshape(128, KB * COUT))

    # base grid (+16 bias for floor correction): cols (k, d, j), n = j*128+p
    p = np.arange(128)
    j = np.arange(NJ)
    n = j[None, :] * 128 + p[:, None]          # [128, 16]
    grids = []
    for half in range(2):
        ho0 = half * HOH
        g = np.empty((128, KK, 2, NJ), np.float32)
        for kh in range(KH):
            for kw in range(KW):
                k = kh * KW + kw
                g[:, k, 0, :] = kh + (ho0 + n // WO) - 1 + 16
                g[:, k, 1, :] = kw + (n % WO) - 1 + 16
        grids.append(np.ascontiguousarray(g.reshape(128, C288)))

    in_maps = []
    for core in range(8):
        b, half = core // 2, core % 2
        ho0 = half * HOH
        offc = offset[b].reshape(KK, 2, HO, WO)[:, :, ho0:ho0 + HOH, :]
        offc = offc.reshape(KK, 2, NJ, 128)          # [k, d, j, p]
        offg_np = np.ascontiguousarray(
            offc.transpose(3, 0, 1, 2).reshape(128, C288))
        in_maps.append({
            "img": imgs[b],
            "offg": offg_np,
            "grid": grids[half],
            "w2": w2,
        })
    return in_maps


def _assemble(results):
    out = np.empty((B, COUT, HO, WO), np.float32)
    for core, r in enumerate(results):
        b, half = core // 2, core % 2
        out[b, :, half * HOH:(half + 1) * HOH, :] = (
            r["out"].reshape(COUT, HOH, WO))
    return out


def kernel(x, offset, weight):
    from concourse.bass_utils import run_bass_kernel_spmd

    nc = _get_nc()
    in_maps = _prep_core_inputs(x, offset, weight)
    res = run_bass_kernel_spmd(nc, in_maps, core_ids=list(range(8)))
    return _assemble(res.results)


# revision 8
# speedup vs baseline: 1.0888x; 1.0888x over previous
"""Deformable convolution (B=4, C=256, 64x64, COUT=256, 3x3) on 8 trn2 NeuronCores.

Sharding: data-parallel over (batch, output-row-half): core i handles batch i//2,
output rows [32*(i%2), 32*(i%2)+32). Weight replicated.

Device pipeline per core:
  1. index/fraction math from offsets (DVE, fp32, immediate-scalar ops only)
  2. one dma_gather per (tap, n-chunk) from a host-staged "quad" image Q in
     DRAM: Q[y*68+x] = [P[y,x,:], P[y+1,x,:]] (fp16, zero-padded borders), so
     each 2KB gathered element carries the full 2x2 bilinear patch
     [TL BL TR BR] for 256 channels.
  3. bilinear combine on DVE in fp16 2x mode: per-quad multiplies against
     pair-duplicated weights (innermost [1,2] AP keeps the fast mode legal)
     + 3 pairwise adds
  4. XBAR dma_start_transpose (sync engine) flips [n,c] -> [c,n] blocks;
     no PE transposes, no PSUM round trip for columns
  5. fp16 GEMM (K=2304) accumulating in PSUM, fp32 output
"""

import os
import sys

for _p in ("/root/.axon_site", "/root/.axon_site/_ro/trn_rl_repo", "/opt/trn_rl_repo"):
    if os.path.isdir(_p) and _p not in sys.path:
        sys.path.append(_p)

import numpy as np

import concourse.bass as bass
import concourse.bacc as bacc
import concourse.mybir as mybir
from concourse.tile import TileContext

# ---------------------------------------------------------------- constants
B, CIN, H, W = 4, 256, 64, 64
COUT, KH, KW = 256, 3, 3
KK = KH * KW                      # 9 taps
HO = WO = 64
HOH = 32                          # output rows per core
N = HOH * WO                      # 2048 positions per core
NJ = 16                           # 128-blocks of N
NCH = 2                           # gather chunks (h)
NJH = NJ // NCH                   # j' blocks per chunk = 8
NIDX = NJH * 128                  # 1024 indices per gather
PAD = 2                           # zero-pad border of the staged image
Hp = Wp = H + 2 * PAD             # 68
NROW = Hp * Wp                    # 4624 quad rows
QE = 4 * CIN                      # 1024 elements per gathered quad
KB = 2 * KK                       # 18 K-blocks of 128
C288 = KK * 2 * NJ                # 288
C144 = KK * NJ                    # 144
FP16 = mybir.dt.float16
FP32 = mybir.dt.float32
I16 = mybir.dt.int16
I32 = mybir.dt.int32
OP = mybir.AluOpType

_MAX_WAITS = 1


def _split_multiwait_instructions(nc):
    """This walrus build rejects >1 sync wait on one instruction ('Too many
    sync wait commands'); hoist extras onto single-wait EventSemaphore
    instructions inserted just before it."""
    fn = nc.m.functions[0]
    for bb in fn.blocks:
        new_insts = []
        for inst in bb.instructions:
            si = getattr(inst, "sync_info", None)
            if si is not None and si.on_wait and len(si.on_wait) > _MAX_WAITS:
                waits = list(si.on_wait)
                for k, w in enumerate(waits[_MAX_WAITS:]):
                    ev = mybir.InstEventSemaphore(
                        name=f"{inst.name}_wsplit{k}",
                        ins=[],
                        outs=[],
                        sync_info=mybir.SyncInfo(on_wait=[w], on_update=[]),
                    )
                    ev.engine = inst.engine
                    new_insts.append(ev)
                si.on_wait = waits[:_MAX_WAITS]
            new_insts.append(inst)
        bb.instructions[:] = new_insts


# ---------------------------------------------------------------- device kernel
def build_nc(split_waits=True):
    nc = bacc.Bacc()
    img = nc.dram_tensor("img", [NROW, QE // 2], FP16, kind="ExternalInput")
    offg = nc.dram_tensor("offg", [128, C288], FP32, kind="ExternalInput")
    grid = nc.dram_tensor("grid", [128, C288], FP32, kind="ExternalInput")
    w2 = nc.dram_tensor("w2", [128, KB * COUT], FP16, kind="ExternalInput")
    out = nc.dram_tensor("out", [COUT, N], FP32, kind="ExternalOutput")

    # gather source: rows of 2*QE fp16 with stride QE (overlapping x-pairs)
    img_src = bass.AP(img[:].tensor, 0, [[QE // 2, NROW - 1], [1, QE]])

    with TileContext(nc) as tc:
        with (
            tc.tile_pool(name="const", bufs=1) as constp,
            tc.tile_pool(name="small", bufs=1) as smallp,
            tc.tile_pool(name="gath", bufs=4) as gathp,
            tc.tile_pool(name="prod", bufs=1) as prodp,
            tc.tile_pool(name="interpa", bufs=4) as vap,
            tc.tile_pool(name="interpb", bufs=1) as vbp,
            tc.tile_pool(name="cols", bufs=4) as colsp,
            tc.tile_pool(name="osb", bufs=2) as osbp,
            tc.tile_pool(name="pout", bufs=2, space="PSUM") as poutp,
        ):
            # ---- constants (offsets first: the gather path depends on them)
            offg_sb = constp.tile([128, C288], FP32)
            nc.sync.dma_start(offg_sb[:], offg[:])
            grid_sb = constp.tile([128, C288], FP32)
            nc.sync.dma_start(grid_sb[:], grid[:])
            w_sb = constp.tile([128, KB, COUT], FP16)
            nc.sync.dma_start(w_sb[:], w2[:].rearrange("p (kb o) -> p kb o", o=COUT))

            # ---- stage A: sampling positions, fractions, weights, indices
            pp = smallp.tile([128, C288], FP32, tag="pp")
            nc.vector.tensor_tensor(out=pp[:], in0=offg_sb[:], in1=grid_sb[:],
                                    op=OP.add)
            # floor(pp): int-cast rounds-to-nearest on HW but truncates in
            # CoreSim; correct either to floor via (cast > pp) ? cast-1 : cast.
            p_i = smallp.tile([128, C288], I32, tag="pi")
            nc.vector.tensor_copy(out=p_i[:], in_=pp[:])
            p_f = smallp.tile([128, C288], FP32, tag="pf")
            nc.vector.tensor_copy(out=p_f[:], in_=p_i[:])
            gt_t = smallp.tile([128, C288], FP32, tag="gtt")
            nc.vector.tensor_tensor(out=gt_t[:], in0=p_f[:], in1=pp[:],
                                    op=OP.is_gt)
            nc.vector.tensor_tensor(out=p_f[:], in0=p_f[:], in1=gt_t[:],
                                    op=OP.subtract)

            # per-tap (k, d, j) views: y = d0, x = d1 -> [128, 9, 16]
            def yx(t):
                v4 = t[:].rearrange("p (k d j) -> p k d j", d=2, j=NJ)
                return v4[:, :, 0, :], v4[:, :, 1, :]

            pf_y, pf_x = yx(p_f)

            # indices first (the gather stream depends only on these):
            # idx = clamp(y0-14,0,67)*68 + clamp(x0-14,0,66)  (+16 host bias)
            tt_ = smallp.tile([128, C144], FP32, tag="tt")
            ss_ = smallp.tile([128, C144], FP32, tag="ss")
            t3 = tt_[:].rearrange("p (k j) -> p k j", j=NJ)
            s3 = ss_[:].rearrange("p (k j) -> p k j", j=NJ)
            nc.vector.tensor_scalar(out=t3, in0=pf_y, scalar1=-14.0,
                                    scalar2=0.0, op0=OP.add, op1=OP.max)
            nc.vector.tensor_scalar(out=tt_[:], in0=tt_[:], scalar1=67.0,
                                    scalar2=float(Wp), op0=OP.min, op1=OP.mult)
            nc.vector.tensor_scalar(out=s3, in0=pf_x, scalar1=-14.0,
                                    scalar2=0.0, op0=OP.add, op1=OP.max)
            nc.vector.tensor_scalar(out=ss_[:], in0=ss_[:], scalar1=66.0,
                                    scalar2=None, op0=OP.min)
            idxf = smallp.tile([128, C144], FP32, tag="idxf")
            nc.vector.tensor_tensor(out=idxf[:], in0=tt_[:], in1=ss_[:],
                                    op=OP.add)
            idxs = smallp.tile([128, C144], I16, tag="idxs")
            nc.vector.tensor_copy(out=idxs[:], in_=idxf[:])

            # fold [128, (k h j')] -> [16, (a k h j')]: partition group a of
            # idxs becomes a free dim (288B-run descriptors, one DMA per a)
            idxf1 = constp.tile([16, 8, C144], I16)
            for a in range(8):
                nc.sync.dma_start(out=idxf1[:, a, :],
                                  in_=idxs[a * 16:(a + 1) * 16, :])
            # DVE reorder -> [16, (k h j' a)] so each gather's 64-index slice
            # [(k h), (j' a)] is contiguous, then replicate to all 8 groups.
            idx2 = constp.tile([128, 8 * C144], I16)
            i_in = idxf1[:].rearrange("b a (kh j) -> b kh j a", kh=KK * NCH)
            i_out = idx2[0:16].rearrange("b (kh j a) -> b kh j a",
                                         kh=KK * NCH, j=NJH)
            nc.vector.tensor_copy(out=i_out, in_=i_in)
            for lo in (16, 32, 64):
                nc.sync.dma_start(out=idx2[lo:2 * lo], in_=idx2[0:lo])

            # bilinear fractions + weights (after idx: off the gather path)
            fr = smallp.tile([128, C288], FP32, tag="fr")
            nc.vector.tensor_tensor(out=fr[:], in0=pp[:], in1=p_f[:],
                                    op=OP.subtract)
            omfr = smallp.tile([128, C288], FP32, tag="omfr")
            nc.vector.tensor_scalar(out=omfr[:], in0=fr[:], scalar1=-1.0,
                                    scalar2=1.0, op0=OP.mult, op1=OP.add)
            fr_y, fr_x = yx(fr)
            om_y, om_x = yx(omfr)

            # bilinear weights -> w4 [128, (k j q)] fp16, q order (TL,BL,TR,BR)
            w4 = smallp.tile([128, C144 * 4], FP16, tag="w4")
            w4v = w4[:].rearrange("p (k j q) -> p k j q", k=KK, j=NJ)
            nc.vector.tensor_tensor(out=w4v[:, :, :, 0], in0=om_y, in1=om_x,
                                    op=OP.mult)  # TL: (1-ly)(1-lx)
            nc.vector.tensor_tensor(out=w4v[:, :, :, 1], in0=fr_y, in1=om_x,
                                    op=OP.mult)  # BL: ly(1-lx)
            nc.vector.tensor_tensor(out=w4v[:, :, :, 2], in0=om_y, in1=fr_x,
                                    op=OP.mult)  # TR: (1-ly)lx
            nc.vector.tensor_tensor(out=w4v[:, :, :, 3], in0=fr_y, in1=fr_x,
                                    op=OP.mult)  # BR: ly lx
            # pair-duplicated weights: innermost [1,2] AP keeps DVE 2x legal
            w4d = smallp.tile([128, C144 * 8], FP16, tag="w4d")
            wv = w4[:]
            w4_rep = bass.AP(wv.tensor, wv.offset,
                             [wv.ap[0], [1, C144 * 4], [0, 2]])
            nc.vector.tensor_copy(
                out=w4d[:].rearrange("p (f two) -> p f two", two=2),
                in_=w4_rep)

            # ---- stages B-E per (h, k)
            for h in range(NCH):
                pout = [poutp.tile([128, NIDX], FP32, tag=f"pout{ob}",
                                   name=f"pout{ob}_{h}")
                        for ob in range(2)]
                for k in range(KK):
                    g = gathp.tile([128, NJH, QE], FP16, tag="g")
                    nc.gpsimd.dma_gather(
                        g[:], img_src,
                        idx2[:, (k * NCH + h) * 64:(k * NCH + h + 1) * 64],
                        NIDX, NIDX, QE, elem_step=QE // 2)

                    # per-quad multiply, fp16 2x (all APs innermost stride 1)
                    prods = prodp.tile([128, NJH, 4, CIN], FP16, tag="prods")
                    g4 = g[:].rearrange("p a (q c) -> p a q c", q=4)
                    for q in range(4):
                        wd = w4d[:, k * 128 + h * 64 + q * 2:]
                        w_q = bass.AP(
                            wd.tensor, wd.offset,
                            [wd.ap[0], [8, NJH], [0, CIN // 2], [1, 2]])
                        nc.vector.tensor_tensor(out=prods[:, :, q, :],
                                                in0=g4[:, :, q, :], in1=w_q,
                                                op=OP.mult)
                    # v layout [p, cb, j', c128] so the per-cb slice is a 2D
                    # contiguous [128, 1024] block for the XBAR transpose
                    v1 = vap.tile([128, 2, NJH, 128], FP16, tag="va")
                    v2 = vbp.tile([128, 2, NJH, 128], FP16, tag="vb")

                    def pq(q):
                        s = prods[:, :, q, :]
                        return s.rearrange("p a (cb c) -> p cb a c", cb=2)

                    nc.vector.tensor_tensor(out=v1[:], in0=pq(0), in1=pq(1),
                                            op=OP.add)
                    nc.vector.tensor_tensor(out=v2[:], in0=pq(2), in1=pq(3),
                                            op=OP.add)
                    nc.vector.tensor_tensor(out=v1[:], in0=v1[:], in1=v2[:],
                                            op=OP.add)

                    # one XBAR transpose for both cb halves:
                    # [n, (cb j' c)] -> [c, (cb j'), n]
                    cols = colsp.tile([128, 2 * NJH, 128], FP16, tag="cols")
                    nc.sync.dma_start_transpose(
                        out=cols[:],
                        in_=v1[:].rearrange("p cb a c -> p (cb a c)"))
                    for cb in range(2):
                        cols_f = cols[:, cb * NJH:(cb + 1) * NJH].rearrange(
                            "p a c -> p (a c)")
                        kb = k * 2 + cb
                        for ob in range(2):
                            for ns in range(2):
                                nc.tensor.matmul(
                                    pout[ob][:, ns * 512:(ns + 1) * 512],
                                    lhsT=w_sb[:, kb, ob * 128:(ob + 1) * 128],
                                    rhs=cols_f[:, ns * 512:(ns + 1) * 512],
                                    start=(kb == 0), stop=(kb == KB - 1))

                for ob in range(2):
                    osb = osbp.tile([128, NIDX], FP32, tag="osb")
                    nc.scalar.copy(out=osb[:], in_=pout[ob][:])
                    nc.sync.dma_start(
                        out=out[ob * 128:(ob + 1) * 128,
                                h * NIDX:(h + 1) * NIDX],
                        in_=osb[:])

    nc.compile()
    if split_waits:
        _split_multiwait_instructions(nc)
    return nc


_NC_CACHE = None


def _get_nc():
    global _NC_CACHE
    if _NC_CACHE is None:
        _NC_CACHE = build_nc()
    return _NC_CACHE


# ---------------------------------------------------------------- host prep
def _prep_core_inputs(x, offset, weight):
    """Build the 8 per-core input maps (pure layout/pad/cast transforms)."""
    x = np.asarray(x, np.float32)
    offset = np.asarray(offset, np.float32)
    weight = np.asarray(weight, np.float32)

    imgs = []
    for b in range(B):
        pimg = np.zeros((Hp + 1, Wp, CIN), np.float16)
        pimg[PAD:PAD + H, PAD:PAD + W, :] = x[b].transpose(1, 2, 0)
        # quad rows: Q[y*68+x] = [P[y,x,:], P[y+1,x,:]]
        quad = np.concatenate([pimg[:Hp], pimg[1:Hp + 1]], axis=2)
        imgs.append(np.ascontiguousarray(quad.reshape(NROW, QE // 2)))

    # dense per-partition weight layout: w2[p, kb, o] = w[o, cb*128+p, kh, kw]
    wT = weight.transpose(2, 3, 1, 0).reshape(KB, 128, COUT).astype(np.float16)
    w2 = np.ascontiguousarray(wT.transpose(1, 0, 2).reshape(128, KB * COUT))

    # base grid (+16 bias for floor correction): cols (k, d, j), n = j*128+p
    p = np.arange(128)
    j = np.arange(NJ)
    n = j[None, :] * 128 + p[:, None]          # [128, 16]
    grids = []
    for half in range(2):
        ho0 = half * HOH
        g = np.empty((128, KK, 2, NJ), np.float32)
        for kh in range(KH):
            for kw in range(KW):
                k = kh * KW + kw
                g[:, k, 0, :] = kh + (ho0 + n // WO) - 1 + 16
                g[:, k, 1, :] = kw + (n % WO) - 1 + 16
        grids.append(np.ascontiguousarray(g.reshape(128, C288)))

    in_maps = []
    for core in range(8):
        b, half = core // 2, core % 2
        ho0 = half * HOH
        offc = offset[b].reshape(KK, 2, HO, WO)[:, :, ho0:ho0 + HOH, :]
        offc = offc.reshape(KK, 2, NJ, 128)          # [k, d, j, p]
        offg_np = np.ascontiguousarray(
            offc.transpose(3, 0, 1, 2).reshape(128, C288))
        in_maps.append({
            "img": imgs[b],
            "offg": offg_np,
            "grid": grids[half],
            "w2": w2,
        })
    return in_maps


def _assemble(results):
    out = np.empty((B, COUT, HO, WO), np.float32)
    for core, r in enumerate(results):
        b, half = core // 2, core % 2
        out[b, :, half * HOH:(half + 1) * HOH, :] = (
            r["out"].reshape(COUT, HOH, WO))
    return out


def kernel(x, offset, weight):
    from concourse.bass_utils import run_bass_kernel_spmd

    nc = _get_nc()
    in_maps = _prep_core_inputs(x, offset, weight)
    res = run_bass_kernel_spmd(nc, in_maps, core_ids=list(range(8)))
    return _assemble(res.results)


# revision 13
# speedup vs baseline: 1.8483x; 1.6977x over previous
"""Deformable convolution (B=4, C=256, 64x64, COUT=256, 3x3) on 8 trn2 NeuronCores.

Sharding: data-parallel over (batch, output-row-half): core i handles batch i//2,
output rows [32*(i%2), 32*(i%2)+32). Weight replicated.

Device pipeline per core:
  1. index/fraction math from offsets (DVE, fp32, immediate-scalar ops only)
  2. one dma_gather per (tap, n-chunk) from a host-staged "quad" image Q in
     DRAM: Q[y*68+x] = [P[y,x,:], P[y+1,x,:]] (fp16, zero-padded borders), so
     each 2KB gathered element carries the full 2x2 bilinear patch
     [TL BL TR BR] for 256 channels.
  3. bilinear combine on DVE in fp16 2x mode: per-quad multiplies against
     pair-duplicated weights (innermost [1,2] AP keeps the fast mode legal)
     + 3 pairwise adds
  4. PE transposes columns to [C,N]-major, ACT copies PSUM->SBUF (the DMA
     xbar transpose cannot be used: it mode-switches the crossbar and the
     scheduler serializes it against the in-flight gather stream)
  5. fp16 GEMM (K=2304) accumulating in PSUM, fp32 output
"""

import os
import sys

for _p in ("/root/.axon_site", "/root/.axon_site/_ro/trn_rl_repo", "/opt/trn_rl_repo"):
    if os.path.isdir(_p) and _p not in sys.path:
        sys.path.append(_p)

import numpy as np

import concourse.bass as bass
import concourse.bacc as bacc
import concourse.mybir as mybir
from concourse.tile import TileContext

# ---------------------------------------------------------------- constants
B, CIN, H, W = 4, 256, 64, 64
COUT, KH, KW = 256, 3, 3
KK = KH * KW                      # 9 taps
HO = WO = 64
HOH = 32                          # output rows per core
N = HOH * WO                      # 2048 positions per core
NJ = 16                           # 128-blocks of N
NCH = 2                           # gather chunks (h)
NJH = NJ // NCH                   # j' blocks per chunk = 8
NIDX = NJH * 128                  # 1024 indices per gather
PAD = 2                           # zero-pad border of the staged image
Hp = Wp = H + 2 * PAD             # 68
NROW = Hp * Wp                    # 4624 quad rows
QE = 4 * CIN                      # 1024 elements per gathered quad
KB = 2 * KK                       # 18 K-blocks of 128
C288 = KK * 2 * NJ                # 288
C144 = KK * NJ                    # 144
FP16 = mybir.dt.float16
FP32 = mybir.dt.float32
I16 = mybir.dt.int16
I32 = mybir.dt.int32
OP = mybir.AluOpType

_MAX_WAITS = 1


def _split_multiwait_instructions(nc):
    """This walrus build rejects >1 sync wait on one instruction ('Too many
    sync wait commands'); hoist extras onto single-wait EventSemaphore
    instructions inserted just before it."""
    fn = nc.m.functions[0]
    for bb in fn.blocks:
        new_insts = []
        for inst in bb.instructions:
            si = getattr(inst, "sync_info", None)
            if si is not None and si.on_wait and len(si.on_wait) > _MAX_WAITS:
                waits = list(si.on_wait)
                for k, w in enumerate(waits[_MAX_WAITS:]):
                    ev = mybir.InstEventSemaphore(
                        name=f"{inst.name}_wsplit{k}",
                        ins=[],
                        outs=[],
                        sync_info=mybir.SyncInfo(on_wait=[w], on_update=[]),
                    )
                    ev.engine = inst.engine
                    new_insts.append(ev)
                si.on_wait = waits[:_MAX_WAITS]
            new_insts.append(inst)
        bb.instructions[:] = new_insts


# ---------------------------------------------------------------- device kernel
def build_nc(split_waits=True):
    nc = bacc.Bacc()
    img = nc.dram_tensor("img", [NROW, QE // 2], FP16, kind="ExternalInput")
    offg = nc.dram_tensor("offg", [128, C288], FP32, kind="ExternalInput")
    grid = nc.dram_tensor("grid", [128, C288], FP32, kind="ExternalInput")
    w2 = nc.dram_tensor("w2", [128, KB * COUT], FP16, kind="ExternalInput")
    ident = nc.dram_tensor("ident", [128, 128], FP16, kind="ExternalInput")
    out = nc.dram_tensor("out", [COUT, N], FP32, kind="ExternalOutput")

    # gather source: rows of 2*QE fp16 with stride QE (overlapping x-pairs)
    img_src = bass.AP(img[:].tensor, 0, [[QE // 2, NROW - 1], [1, QE]])

    with TileContext(nc) as tc:
        with (
            tc.tile_pool(name="const", bufs=1) as constp,
            tc.tile_pool(name="small", bufs=1) as smallp,
            tc.tile_pool(name="gath", bufs=4) as gathp,
            tc.tile_pool(name="prod", bufs=1) as prodp,
            tc.tile_pool(name="interpa", bufs=4) as vap,
            tc.tile_pool(name="interpb", bufs=1) as vbp,
            tc.tile_pool(name="cols", bufs=4) as colsp,
            tc.tile_pool(name="osb", bufs=2) as osbp,
            tc.tile_pool(name="pt", bufs=3, space="PSUM") as ptp,
            tc.tile_pool(name="pout", bufs=1, space="PSUM") as poutp,
        ):
            # ---- constants (offsets first: the gather path depends on them)
            offg_sb = constp.tile([128, C288], FP32)
            nc.sync.dma_start(offg_sb[:], offg[:])
            grid_sb = constp.tile([128, C288], FP32)
            nc.sync.dma_start(grid_sb[:], grid[:])
            id_sb = constp.tile([128, 128], FP16)
            nc.sync.dma_start(id_sb[:], ident[:])
            w_sb = constp.tile([128, KB, COUT], FP16)
            nc.sync.dma_start(w_sb[:], w2[:].rearrange("p (kb o) -> p kb o", o=COUT))

            # ---- stage A: sampling positions, fractions, weights, indices
            pp = smallp.tile([128, C288], FP32, tag="pp")
            nc.vector.tensor_tensor(out=pp[:], in0=offg_sb[:], in1=grid_sb[:],
                                    op=OP.add)
            # floor(pp): int-cast rounds-to-nearest on HW but truncates in
            # CoreSim; correct either to floor via (cast > pp) ? cast-1 : cast.
            p_i = smallp.tile([128, C288], I32, tag="pi")
            nc.vector.tensor_copy(out=p_i[:], in_=pp[:])
            p_f = smallp.tile([128, C288], FP32, tag="pf")
            nc.vector.tensor_copy(out=p_f[:], in_=p_i[:])
            gt_t = smallp.tile([128, C288], FP32, tag="gtt")
            nc.vector.tensor_tensor(out=gt_t[:], in0=p_f[:], in1=pp[:],
                                    op=OP.is_gt)
            nc.vector.tensor_tensor(out=p_f[:], in0=p_f[:], in1=gt_t[:],
                                    op=OP.subtract)

            # per-tap (k, d, j) views: y = d0, x = d1 -> [128, 9, 16]
            def yx(t):
                v4 = t[:].rearrange("p (k d j) -> p k d j", d=2, j=NJ)
                return v4[:, :, 0, :], v4[:, :, 1, :]

            pf_y, pf_x = yx(p_f)

            # indices first (the gather stream depends only on these):
            # idx = clamp(y0-14,0,67)*68 + clamp(x0-14,0,66)  (+16 host bias)
            tt_ = smallp.tile([128, C144], FP32, tag="tt")
            ss_ = smallp.tile([128, C144], FP32, tag="ss")
            t3 = tt_[:].rearrange("p (k j) -> p k j", j=NJ)
            s3 = ss_[:].rearrange("p (k j) -> p k j", j=NJ)
            nc.vector.tensor_scalar(out=t3, in0=pf_y, scalar1=-14.0,
                                    scalar2=0.0, op0=OP.add, op1=OP.max)
            nc.vector.tensor_scalar(out=tt_[:], in0=tt_[:], scalar1=67.0,
                                    scalar2=float(Wp), op0=OP.min, op1=OP.mult)
            nc.vector.tensor_scalar(out=s3, in0=pf_x, scalar1=-14.0,
                                    scalar2=0.0, op0=OP.add, op1=OP.max)
            nc.vector.tensor_scalar(out=ss_[:], in0=ss_[:], scalar1=66.0,
                                    scalar2=None, op0=OP.min)
            idxf = smallp.tile([128, C144], FP32, tag="idxf")
            nc.vector.tensor_tensor(out=idxf[:], in0=tt_[:], in1=ss_[:],
                                    op=OP.add)
            idxs = smallp.tile([128, C144], I16, tag="idxs")
            nc.vector.tensor_copy(out=idxs[:], in_=idxf[:])

            # fold [128, (k h j')] -> [16, (a k h j')]: partition group a of
            # idxs becomes a free dim (288B-run descriptors, one DMA per a)
            idxf1 = constp.tile([16, 8, C144], I16)
            for a in range(8):
                nc.sync.dma_start(out=idxf1[:, a, :],
                                  in_=idxs[a * 16:(a + 1) * 16, :])
            # DVE reorder -> [16, (k h j' a)] so each gather's 64-index slice
            # [(k h), (j' a)] is contiguous, then replicate to all 8 groups.
            idx2 = constp.tile([128, 8 * C144], I16)
            i_in = idxf1[:].rearrange("b a (kh j) -> b kh j a", kh=KK * NCH)
            i_out = idx2[0:16].rearrange("b (kh j a) -> b kh j a",
                                         kh=KK * NCH, j=NJH)
            nc.vector.tensor_copy(out=i_out, in_=i_in)
            for lo in (16, 32, 64):
                nc.sync.dma_start(out=idx2[lo:2 * lo], in_=idx2[0:lo])

            # bilinear fractions + weights (after idx: off the gather path)
            fr = smallp.tile([128, C288], FP32, tag="fr")
            nc.vector.tensor_tensor(out=fr[:], in0=pp[:], in1=p_f[:],
                                    op=OP.subtract)
            omfr = smallp.tile([128, C288], FP32, tag="omfr")
            nc.vector.tensor_scalar(out=omfr[:], in0=fr[:], scalar1=-1.0,
                                    scalar2=1.0, op0=OP.mult, op1=OP.add)
            fr_y, fr_x = yx(fr)
            om_y, om_x = yx(omfr)

            # bilinear weights -> w4 [128, (k j q)] fp16, q order (TL,BL,TR,BR)
            w4 = smallp.tile([128, C144 * 4], FP16, tag="w4")
            w4v = w4[:].rearrange("p (k j q) -> p k j q", k=KK, j=NJ)
            nc.vector.tensor_tensor(out=w4v[:, :, :, 0], in0=om_y, in1=om_x,
                                    op=OP.mult)  # TL: (1-ly)(1-lx)
            nc.vector.tensor_tensor(out=w4v[:, :, :, 1], in0=fr_y, in1=om_x,
                                    op=OP.mult)  # BL: ly(1-lx)
            nc.vector.tensor_tensor(out=w4v[:, :, :, 2], in0=om_y, in1=fr_x,
                                    op=OP.mult)  # TR: (1-ly)lx
            nc.vector.tensor_tensor(out=w4v[:, :, :, 3], in0=fr_y, in1=fr_x,
                                    op=OP.mult)  # BR: ly lx
            # pair-duplicated weights: innermost [1,2] AP keeps DVE 2x legal
            w4d = smallp.tile([128, C144 * 8], FP16, tag="w4d")
            wv = w4[:]
            w4_rep = bass.AP(wv.tensor, wv.offset,
                             [wv.ap[0], [1, C144 * 4], [0, 2]])
            nc.vector.tensor_copy(
                out=w4d[:].rearrange("p (f two) -> p f two", two=2),
                in_=w4_rep)

            # ---- stages B-E per (h, k)
            for h in range(NCH):
                pout = [poutp.tile([128, NIDX], FP32, tag=f"pout{ob}",
                                   name=f"pout{ob}_{h}")
                        for ob in range(2)]
                for k in range(KK):
                    g = gathp.tile([128, NJH, QE], FP16, tag="g")
                    nc.gpsimd.dma_gather(
                        g[:], img_src,
                        idx2[:, (k * NCH + h) * 64:(k * NCH + h + 1) * 64],
                        NIDX, NIDX, QE, elem_step=QE // 2)

                    # per-quad multiply, fp16 2x (all APs innermost stride 1)
                    prods = prodp.tile([128, NJH, 4, CIN], FP16, tag="prods")
                    g4 = g[:].rearrange("p a (q c) -> p a q c", q=4)
                    for q in range(4):
                        wd = w4d[:, k * 128 + h * 64 + q * 2:]
                        w_q = bass.AP(
                            wd.tensor, wd.offset,
                            [wd.ap[0], [8, NJH], [0, CIN // 2], [1, 2]])
                        nc.vector.tensor_tensor(out=prods[:, :, q, :],
                                                in0=g4[:, :, q, :], in1=w_q,
                                                op=OP.mult)
                    # v layout [p, cb, j', c128] so the per-cb slice is a 2D
                    # contiguous [128, 1024] block for the XBAR transpose
                    v1 = vap.tile([128, 2, NJH, 128], FP16, tag="va")
                    v2 = vbp.tile([128, 2, NJH, 128], FP16, tag="vb")

                    def pq(q):
                        s = prods[:, :, q, :]
                        return s.rearrange("p a (cb c) -> p cb a c", cb=2)

                    nc.vector.tensor_tensor(out=v1[:], in0=pq(0), in1=pq(1),
                                            op=OP.add)
                    nc.vector.tensor_tensor(out=v2[:], in0=pq(2), in1=pq(3),
                                            op=OP.add)
                    nc.vector.tensor_tensor(out=v1[:], in0=v1[:], in1=v2[:],
                                            op=OP.add)

                    for cb in range(2):
                        pt = ptp.tile([128, NJH, 128], FP16, tag="pt")
                        for j in range(NJH):
                            nc.tensor.transpose(
                                pt[:, j, :], v1[:, cb, j, :], id_sb[:])
                        cols = colsp.tile([128, NIDX], FP16, tag="cols")
                        nc.scalar.copy(out=cols[:],
                                       in_=pt[:].rearrange("p a b -> p (a b)"))
                        kb = k * 2 + cb
                        for ob in range(2):
                            for ns in range(2):
                                nc.tensor.matmul(
                                    pout[ob][:, ns * 512:(ns + 1) * 512],
                                    lhsT=w_sb[:, kb, ob * 128:(ob + 1) * 128],
                                    rhs=cols[:, ns * 512:(ns + 1) * 512],
                                    start=(kb == 0), stop=(kb == KB - 1))

                for ob in range(2):
                    osb = osbp.tile([128, NIDX], FP32, tag="osb")
                    nc.scalar.copy(out=osb[:], in_=pout[ob][:])
                    nc.sync.dma_start(
                        out=out[ob * 128:(ob + 1) * 128,
                                h * NIDX:(h + 1) * NIDX],
                        in_=osb[:])

    nc.compile()
    if split_waits:
        _split_multiwait_instructions(nc)
    return nc


_NC_CACHE = None


def _get_nc():
    global _NC_CACHE
    if _NC_CACHE is None:
        _NC_CACHE = build_nc()
    return _NC_CACHE


# ---------------------------------------------------------------- host prep
def _prep_core_inputs(x, offset, weight):
    """Build the 8 per-core input maps (pure layout/pad/cast transforms)."""
    x = np.asarray(x, np.float32)
    offset = np.asarray(offset, np.float32)
    weight = np.asarray(weight, np.float32)

    imgs = []
    for b in range(B):
        pimg = np.zeros((Hp + 1, Wp, CIN), np.float16)
        pimg[PAD:PAD + H, PAD:PAD + W, :] = x[b].transpose(1, 2, 0)
        # quad rows: Q[y*68+x] = [P[y,x,:], P[y+1,x,:]]
        quad = np.concatenate([pimg[:Hp], pimg[1:Hp + 1]], axis=2)
        imgs.append(np.ascontiguousarray(quad.reshape(NROW, QE // 2)))

    # dense per-partition weight layout: w2[p, kb, o] = w[o, cb*128+p, kh, kw]
    wT = weight.transpose(2, 3, 1, 0).reshape(KB, 128, COUT).astype(np.float16)
    w2 = np.ascontiguousarray(wT.transpose(1, 0, 2).reshape(128, KB * COUT))

    # base grid (+16 bias for floor correction): cols (k, d, j), n = j*128+p
    p = np.arange(128)
    j = np.arange(NJ)
    n = j[None, :] * 128 + p[:, None]          # [128, 16]
    grids = []
    for half in range(2):
        ho0 = half * HOH
        g = np.empty((128, KK, 2, NJ), np.float32)
        for kh in range(KH):
            for kw in range(KW):
                k = kh * KW + kw
                g[:, k, 0, :] = kh + (ho0 + n // WO) - 1 + 16
                g[:, k, 1, :] = kw + (n % WO) - 1 + 16
        grids.append(np.ascontiguousarray(g.reshape(128, C288)))

    in_maps = []
    for core in range(8):
        b, half = core // 2, core % 2
        ho0 = half * HOH
        offc = offset[b].reshape(KK, 2, HO, WO)[:, :, ho0:ho0 + HOH, :]
        offc = offc.reshape(KK, 2, NJ, 128)          # [k, d, j, p]
        offg_np = np.ascontiguousarray(
            offc.transpose(3, 0, 1, 2).reshape(128, C288))
        in_maps.append({
            "img": imgs[b],
            "offg": offg_np,
            "grid": grids[half],
            "w2": w2,
            "ident": np.eye(128, dtype=np.float16),
        })
    return in_maps


def _assemble(results):
    out = np.empty((B, COUT, HO, WO), np.float32)
    for core, r in enumerate(results):
        b, half = core // 2, core % 2
        out[b, :, half * HOH:(half + 1) * HOH, :] = (
            r["out"].reshape(COUT, HOH, WO))
    return out


def kernel(x, offset, weight):
    from concourse.bass_utils import run_bass_kernel_spmd

    nc = _get_nc()
    in_maps = _prep_core_inputs(x, offset, weight)
    res = run_bass_kernel_spmd(nc, in_maps, core_ids=list(range(8)))
    return _assemble(res.results)
